# revision 72
# baseline (speedup 1.0000x reference)
"""Multi-head attention (B=4, N=2048, C=1024, H=16, D=64) on 8 Trainium2 cores.

Sharding: tensor-parallel over heads (2 heads per core); every core reads the
full x (pre-transposed to [C, tokens] bf16 on host) plus its head-slice of the
weights, and writes a full-shape partial projection output.  The 8 partials
are summed on host (the "unshard" of a row-parallel output projection).

v3 layout (372us baseline -> 310us): restructured around the two
near-saturated engines, ACT (exp of 33.5M scores/core) and PE.

  - exp batching: the two heads' score tiles for one key chunk land in one
    2-bank PSUM tile [128, 2, 512] f32, so each ACT instruction processes
    1024 elements (amortizes the fixed SBUF/PSUM access latency); ACT busy
    drops 313us -> ~266us and is the pipeline's pace-setter.
  - AV in the flipped orientation: probs are the *stationary* operand
    (pb[:, hh, 128q-slice]), V' [128k, 65] moves, out = av[128q, 65] f32
    accumulated over the 16 key chunks.  Cost per matmul is the moving free
    size (65) instead of 512, halving AV PE time; the ones-column denominator
    rides along as column 64 (scaled by S2 to match the fp8 product scale).
  - normalization: the denominator is per-query on the PARTITION dim, so a
    [128,1] reciprocal_approx_fast + per-partition tensor_scalar_mul
    replaces the PE broadcast matmul + row shuffle of the baseline.
  - the projection operand attn^T [ch, q] is produced by PE transpose
    (128-cycle instrs) + DVE copies instead of re-orienting AV.
  - phase A runs 3-term split-precision fp8 DoubleRow matmuls
    (x = xh + xl, w = wh + wl, hi*hi + lo*hi + hi*lo at 0.5 PE
    cycles/row), ~25% less PE time than bf16 at slightly BETTER accuracy;
    planes are pre-scaled/pre-swizzled on host so every DMA is one
    contiguous run per partition.
  - emission is software-pipelined per (batch, query-block) slot: slot s
    emits ST/exp of slot s+1 paced 1:1 against everything else, with phase-A
    fill (exp-independent) front-loaded and the exp-dependent
    AV->norm->transpose->projection chain in the back half.  All x/weight
    DMAs ride the SP queue (the scalar queue shares the ACT sequencer and
    would head-block the exp stream); y goes out once per slot on the Pool
    SWDGE queue.

Score/AV/projection matmuls are bf16 (1 PE cycle/row) with fp32 PSUM
accumulation.  Cost-model sim: ~310 us/core (PE ~259us busy, ACT ~267us).
"""

import numpy as np
import ml_dtypes
from contextlib import ExitStack

import concourse.mybir as mybir
import concourse.tile as tile
from concourse import bacc
from concourse import bass_utils
from concourse.masks import make_identity

B, N, C = 4, 2048, 1024
H, D = 16, 64
T = B * N                 # 8192 tokens
NCORES = 8
HPC = H // NCORES         # heads per core = 2
SCALE = D ** -0.5

F32 = mybir.dt.float32
BF16 = mybir.dt.bfloat16

TS = 512                  # phase-A token tile (free dim)
CCN = C // 128            # 8 contraction chunks
KC = N // 128             # 16 key chunks per batch
QB = N // 512             # 4 query blocks per batch
NSLOT = B * QB            # 16 (b, qb) slots


FP8 = mybir.dt.float8e4
FP8A = True               # fp8 split-precision phase A (see phase_A_qk)
XS = 4.0 if FP8A else 1.0  # host-side scale on x planes
WS = 16.0 if FP8A else 1.0  # host-side scale on wqk/wv planes
S2 = XS * WS              # scale carried by qT/kT/v
XDT = FP8 if FP8A else mybir.dt.bfloat16


def _build_graph():
    nc = bacc.Bacc("TRN2", target_bir_lowering=False, debug=False,
                   num_devices=NCORES)
    # x and the qkv weights ship as split-precision fp8 pairs (hi + residual);
    # phase A runs 3-term DoubleRow matmuls (hi*hi + lo*hi + hi*lo) at 2x the
    # bf16 matmul rate with ~bf16 accuracy.  x is scaled by XS and w by WS on
    # host so the residual planes stay in fp8's normal range; the product
    # scale S2 rides on qT/kT/v and is folded into the exp scale and the
    # ones-column (= S2) of V'.
    # all fp8 planes ship pre-swizzled to [partition, ...contiguous] so each
    # DMA is one contiguous run per partition (descriptor overhead killed)
    NTS = T // TS
    xh = nc.dram_tensor("xh", [128, NTS, CCN, TS], XDT, kind="ExternalInput").ap()
    # wqk columns: [q_h0 | q_h1 | k_h0 | k_h1], each D wide
    wqkh = nc.dram_tensor("wqkh", [128, CCN, HPC * 2 * D], XDT, kind="ExternalInput").ap()
    if FP8A:
        xl = nc.dram_tensor("xl", [128, NTS, CCN, TS], FP8, kind="ExternalInput").ap()
        wqkl = nc.dram_tensor("wqkl", [128, CCN, HPC * 2 * D], FP8, kind="ExternalInput").ap()
        wvl = nc.dram_tensor("wvl", [128, CCN, HPC * D], FP8, kind="ExternalInput").ap()
    wvh = nc.dram_tensor("wvh", [128, CCN, HPC * D], XDT, kind="ExternalInput").ap()
    wp = nc.dram_tensor("wp", [HPC * D, C], BF16, kind="ExternalInput").ap()
    y = nc.dram_tensor("y", [T, C], BF16, kind="ExternalOutput").ap()

    with tile.TileContext(nc) as tc, ExitStack() as ctx:
        const = ctx.enter_context(tc.tile_pool(name="const", bufs=1))
        xpool = ctx.enter_context(tc.tile_pool(name="x", bufs=6))
        probs = ctx.enter_context(tc.tile_pool(name="probs", bufs=34))
        attnp = ctx.enter_context(tc.tile_pool(name="attn", bufs=6))
        atTp = ctx.enter_context(tc.tile_pool(name="attnT", bufs=3))
        rpool = ctx.enter_context(tc.tile_pool(name="recip", bufs=4))
        outp = ctx.enter_context(tc.tile_pool(name="out", bufs=2))
        # PSUM: st 2x2 banks + av 2x1 + mm 2x1 = 8 banks
        st_psum = ctx.enter_context(
            tc.tile_pool(name="st", bufs=2, space="PSUM"))
        av_psum = ctx.enter_context(
            tc.tile_pool(name="av", bufs=2, space="PSUM"))
        mm_psum = ctx.enter_context(
            tc.tile_pool(name="mm", bufs=2, space="PSUM"))

        ident = const.tile([128, 128], BF16)
        make_identity(nc, ident[:])
        # burn through the PE p-state ramp while the first DMAs land
        for _ in range(28):
            warm = mm_psum.tile([128, 128], BF16, name="warm", tag="mm")
            nc.tensor.transpose(warm[:], ident[:], ident[:])

        wqkh_sb = const.tile([128, CCN, HPC * 2 * D], XDT)
        wvh_sb = const.tile([128, CCN, HPC * D], XDT)
        if FP8A:
            wqkl_sb = const.tile([128, CCN, HPC * 2 * D], FP8)
            wvl_sb = const.tile([128, CCN, HPC * D], FP8)
        wp_sb = const.tile([128, C], BF16)

        # per-batch phase-A outputs:
        #   qT/kT: [d, tokens] * S2; head hh lives on partitions hh*64..+63
        #   v: per head, 16 key-tiles of [128 tok, 65] * S2 (col 64 = S2)
        qT_b, kT_b, v_b = [], [], []
        for b in range(B):
            qT_b.append(const.tile([128, N], BF16, name=f"qTb{b}", tag=f"qT{b}"))
            kT_b.append(const.tile([128, N], BF16, name=f"kTb{b}", tag=f"kT{b}"))
            v_b.append(const.tile([128, HPC, KC, D + 1], BF16,
                                  name=f"vb{b}", tag=f"v{b}"))
            nc.gpsimd.memset(v_b[b][:, :, :, D:D + 1], S2)

        x_tiles = {}

        def phase_A_dma(b, ts):
            ci = b * 4 + ts
            xt_h = xpool.tile([128, CCN, TS], XDT, name=f"xh{b}_{ts}", tag="xth")
            # all x planes on the SP queue: the scalar queue shares the ACT
            # sequencer and would head-block the exp stream
            nc.sync.dma_start(xt_h[:], xh[:, ci, :, :])
            xt_l = None
            if FP8A:
                xt_l = xpool.tile([128, CCN, TS], FP8,
                                  name=f"xl{b}_{ts}", tag="xtl")
                nc.sync.dma_start(xt_l[:], xl[:, ci, :, :])
            x_tiles[(b, ts)] = (xt_h, xt_l)

        DR = mybir.MatmulPerfMode.DoubleRow

        def phase_A_qk(b, ts, qk_i):
            # one of two units per (b, ts): dst in {q, k}; the accumulation
            # group stays within the unit so the mm pool never rotates under
            # an open accumulator
            xt_h, xt_l = x_tiles[(b, ts)]
            dst = (qT_b[b], kT_b[b])[qk_i]
            csl = slice(qk_i * 128, (qk_i + 1) * 128)
            ps = mm_psum.tile([128, TS], F32, name=f"ps{b}_{ts}_{qk_i}",
                              tag="mm")
            if FP8A:
                terms = ((xt_h, wqkh_sb), (xt_l, wqkh_sb), (xt_h, wqkl_sb))
                for ti in range(3):
                    xt, wt = terms[ti]
                    for cp in range(4):
                        nc.tensor.matmul(
                            ps[:],
                            wt[:, 2 * cp:2 * cp + 2, csl],
                            xt[:, 2 * cp:2 * cp + 2, :],
                            start=(ti == 0 and cp == 0),
                            stop=(ti == 2 and cp == 3),
                            perf_mode=DR)
            else:
                for cc in range(CCN):
                    nc.tensor.matmul(
                        ps[:], wqkh_sb[:, cc, csl], xt_h[:, cc, :],
                        start=(cc == 0), stop=(cc == CCN - 1))
            nc.vector.tensor_copy(dst[:, ts * TS:(ts + 1) * TS], ps[:])

        def phase_A_v(b, ts, jh):
            # one of two units per (b, ts): token-subtile pair
            xt_h, xt_l = x_tiles[(b, ts)]
            for j in (2 * jh, 2 * jh + 1):
                jsl = slice(j * 128, (j + 1) * 128)
                vp = mm_psum.tile([128, HPC * D], F32, tag="mm")
                if FP8A:
                    terms = ((xt_h, wvh_sb), (xt_l, wvh_sb), (xt_h, wvl_sb))
                    for ti in range(3):
                        xt, wt = terms[ti]
                        for cp in range(4):
                            nc.tensor.matmul(
                                vp[:],
                                xt[:, 2 * cp:2 * cp + 2, jsl],
                                wt[:, 2 * cp:2 * cp + 2, :],
                                start=(ti == 0 and cp == 0),
                                stop=(ti == 2 and cp == 3),
                                perf_mode=DR)
                else:
                    for cc in range(CCN):
                        nc.tensor.matmul(
                            vp[:], xt_h[:, cc, jsl], wvh_sb[:, cc, :],
                            start=(cc == 0), stop=(cc == CCN - 1))
                for hh in range(HPC):
                    nc.vector.tensor_copy(
                        v_b[b][:, hh, ts * 4 + j, 0:D],
                        vp[:, hh * D:(hh + 1) * D])
            if jh == 1:
                x_tiles.pop((b, ts))

        # ---- phase B building blocks --------------------------------------
        pb_tiles = {}

        def st_exp(s, kc):
            b, qb = divmod(s, QB)
            q0, k0 = qb * 512, kc * 128
            st = st_psum.tile([128, 2, 512], F32, tag="st")
            for hh in range(HPC):
                h0 = hh * 64
                nc.tensor.matmul(
                    st[:, hh, :],
                    kT_b[b][h0:h0 + 64, k0:k0 + 128],
                    qT_b[b][h0:h0 + 64, q0:q0 + 512],
                    start=True, stop=True)
            pb = probs.tile([128, 2, 512], BF16, tag="probs")
            nc.scalar.activation(pb[:], st[:],
                                 mybir.ActivationFunctionType.Exp,
                                 scale=SCALE / (S2 * S2))
            pb_tiles[(s, kc)] = pb

        av_tiles = {}

        def av_group(s, hh, qsub):
            b, qb = divmod(s, QB)
            if qsub == 0:
                av_tiles[(s, hh)] = av_psum.tile([128, 4, D + 1], F32,
                                                 name=f"av{s}_{hh}", tag="av")
            av = av_tiles[(s, hh)]
            for kc in range(KC):
                pb = pb_tiles[(s, kc)]
                nc.tensor.matmul(
                    av[:, qsub, :],
                    pb[:, hh, qsub * 128:(qsub + 1) * 128],
                    v_b[b][:, hh, kc, :],
                    start=(kc == 0), stop=(kc == KC - 1))
            if hh == HPC - 1 and qsub == 3:
                for kc in range(KC):
                    pb_tiles.pop((s, kc))

        def av_kc(s, kc):
            # kc-major variant for the final slot: accumulate all 8 regions
            # per key chunk so AV tracks the exp stream instead of
            # serializing after it
            b, qb = divmod(s, QB)
            if kc == 0:
                for hh in range(HPC):
                    av_tiles[(s, hh)] = av_psum.tile(
                        [128, 4, D + 1], F32, name=f"av{s}_{hh}", tag="av")
            pb = pb_tiles[(s, kc)]
            for hh in range(HPC):
                for qsub in range(4):
                    nc.tensor.matmul(
                        av_tiles[(s, hh)][:, qsub, :],
                        pb[:, hh, qsub * 128:(qsub + 1) * 128],
                        v_b[b][:, hh, kc, :],
                        start=(kc == 0), stop=(kc == KC - 1))
            if kc == KC - 1:
                for k2 in range(KC):
                    pb_tiles.pop((s, k2))

        attnT_tiles = {}
        at_tiles = {}

        def norm_mul(s, qsub):
            # per-query normalization on DVE (denominator = av column 64,
            # carrying the same S2 scale as the numerator)
            if qsub == 0:
                attnT_tiles[s] = atTp.tile([128, 4, 128], BF16,
                                           name=f"aT{s}", tag="attnT")
            rc = rpool.tile([128, 2], F32, name=f"rc{s}_{qsub}", tag="rc")
            at = attnp.tile([128, 128], BF16, tag="attn")
            at_tiles[(s, qsub)] = at
            for hh in range(HPC):
                nc.vector.reciprocal_approx_fast(
                    rc[:, hh:hh + 1], av_tiles[(s, hh)][:, qsub, D:D + 1])
                nc.vector.tensor_scalar_mul(
                    at[:, hh * 64:(hh + 1) * 64],
                    av_tiles[(s, hh)][:, qsub, 0:D],
                    rc[:, hh:hh + 1])

        def norm_T(s, qsub):
            # PE transpose into the projection orientation [ch, q]
            aT = attnT_tiles[s]
            at = at_tiles.pop((s, qsub))
            tp = mm_psum.tile([128, 128], BF16, tag="mm")
            nc.tensor.transpose(tp[:], at[:], ident[:])
            nc.vector.tensor_copy(aT[:, qsub, :], tp[:])

        ot_tiles = {}

        def phase_P(s, qsub):
            b, qb = divmod(s, QB)
            aT = attnT_tiles[s]
            if qsub == 0:
                ot_tiles[s] = outp.tile([128, 4, C], BF16, name=f"ot{s}",
                                        tag="out")
            ot = ot_tiles[s]
            for ob in range(2):
                pp = mm_psum.tile([128, 512], F32, tag="mm")
                nc.tensor.matmul(
                    pp[:],
                    aT[:, qsub, :],
                    wp_sb[:, ob * 512:(ob + 1) * 512],
                    start=True, stop=True)
                nc.vector.tensor_copy(ot[:, qsub, ob * 512:(ob + 1) * 512],
                                      pp[:])
            if qsub == 3:
                # one batched y DMA per slot on the Pool SWDGE queue, keeping
                # the sync HWDGE queue clear for the x streams
                r0 = b * N + qb * 512
                nc.gpsimd.dma_start(
                    y[r0:r0 + 512, :].rearrange("(q p) c -> p q c", p=128),
                    ot[:])
                attnT_tiles.pop(s)
                ot_tiles.pop(s)

        # ---- software-pipelined emission ----------------------------------
        # prologue: batch 0 projections feeding slot 0's ST/exp stream.
        # DMA transfers serialize on one global resource in the cost model,
        # so order them by first use: x chunk 0 and the wqk planes gate the
        # first projection matmul.
        phase_A_dma(0, 0)
        nc.sync.dma_start(wqkh_sb[:], wqkh[:, :, :])
        if FP8A:
            nc.sync.dma_start(wqkl_sb[:], wqkl[:, :, :])
        phase_A_dma(0, 1)
        nc.sync.dma_start(wvh_sb[:], wvh[:, :, :])
        if FP8A:
            nc.sync.dma_start(wvl_sb[:], wvl[:, :, :])
        nc.sync.dma_start(wp_sb[:], wp[:, :])
        # qk-only per ts round (PE ~4.3us vs ACT 4.15us per round); the
        # batch-0 V projections wait until slot 0's fill, where the PE would
        # otherwise idle
        for ts in range(4):
            phase_A_qk(0, ts, 0)
            phase_A_qk(0, ts, 1)
            if ts + 2 < 4:
                phase_A_dma(0, ts + 2)
            st_exp(0, 4 * ts)
            st_exp(0, 4 * ts + 1)
            st_exp(0, 4 * ts + 2)
            st_exp(0, 4 * ts + 3)

        # slot s: ST/exp(s+1) interleaved with the AV->norm->P chain of slot
        # s and phase A of batch b+1.  Emission order IS execution order per
        # engine, so AV groups must follow the A_v writes of their batch and
        # each norm/T must follow both heads' AV groups of its qsub.
        for s in range(NSLOT):
            b, qb = divmod(s, QB)
            nb = b + 1
            sts = ([("st", kc) for kc in range(KC)]
                   if s + 1 < NSLOT else [])
            # per-qsub chain: both heads' AV accumulations, then
            # normalize+transpose, then the output projection; nt/p trail
            # their producers by several units so the PE never stalls on the
            # DVE round-trips (PE executes in emission order)
            chain = [("av", 0, 0), ("av", 1, 0), ("av", 0, 1),
                     ("av", 1, 1), ("nm", s, 0), ("av", 0, 2),
                     ("av", 1, 2), ("nm", s, 1), ("av", 0, 3),
                     ("nt", s, 0), ("av", 1, 3), ("nm", s, 2),
                     ("nt", s, 1), ("p", s, 0), ("nm", s, 3),
                     ("nt", s, 2), ("p", s, 1), ("nt", s, 3),
                     ("p", s, 2), ("p", s, 3)]
            pre = []      # must precede this slot's AV groups
            if s == 0:
                pre += [("av_", 0, ts, jh)
                        for ts in range(4) for jh in range(2)]
            if b >= 1 and qb == 0:
                pre += [("av_", b, 3, 0), ("av_", b, 3, 1)]
            fill = []     # order-flexible PE filler (phase A of next batch)
            if nb < B:
                if qb >= 1:
                    fill += [("av_", nb, qb - 1, 0), ("av_", nb, qb - 1, 1)]
                fill += [("aqk", nb, qb, 0), ("aqk", nb, qb, 1)]
            # Front-load the fill (phase A of the next batch): it does not
            # depend on this slot's exps, so it soaks up the PE idle while
            # ACT drains them; the AV->norm->P chain (which must wait for
            # those exps) runs in the back half, 2 chain units per ST.
            seq = []
            if nb < B:
                seq.append(("dma", nb, qb))
            seq += sts[0:2]
            seq += pre
            si = 2
            for u in fill:
                seq.append(u)
                if si < len(sts):
                    seq.append(sts[si]); si += 1
            ci = 0
            while ci < len(chain) or si < len(sts):
                if ci < len(chain):
                    seq.append(chain[ci]); ci += 1
                if ci < len(chain):
                    seq.append(chain[ci]); ci += 1
                if si < len(sts):
                    seq.append(sts[si]); si += 1

            for u in seq:
                kind = u[0]
                if kind == "st":
                    st_exp(s + 1, u[1])
                elif kind == "av":
                    av_group(s, u[1], u[2])
                elif kind == "avk":
                    av_kc(s, u[1])
                elif kind == "nm":
                    norm_mul(s, u[2])
                elif kind == "nt":
                    norm_T(s, u[2])
                elif kind == "p":
                    phase_P(u[1], u[2])
                elif kind == "dma":
                    phase_A_dma(u[1], u[2])
                elif kind == "aqk":
                    phase_A_qk(u[1], u[2], u[3])
                elif kind == "av_":
                    phase_A_v(u[1], u[2], u[3])

    nc.compile()
    return nc


_NC = None
LAST_EXEC_NS = None


def _get_nc():
    global _NC
    if _NC is None:
        _NC = _build_graph()
    return _NC


def _fp8_split(a, scale):
    """a*scale ~= hi + lo with both planes fp8e4m3."""
    f8 = ml_dtypes.float8_e4m3
    a = a.astype(np.float32) * scale
    hi = a.astype(f8)
    lo = (a - hi.astype(np.float32)).astype(f8)
    return np.ascontiguousarray(hi), np.ascontiguousarray(lo)


def _swz_x(a):
    # [C, T] -> [128, T//TS, CCN, TS]: per-partition contiguous DMA chunks
    return np.ascontiguousarray(
        a.reshape(CCN, 128, T // TS, TS).transpose(1, 2, 0, 3))


def _swz_w(a):
    # [C, W] -> [128, CCN, W]
    return np.ascontiguousarray(
        a.reshape(CCN, 128, a.shape[1]).transpose(1, 0, 2))


def _make_in_maps(x, W_qkv, W_proj):
    bf = ml_dtypes.bfloat16
    xT = x.reshape(T, C).T
    if FP8A:
        xh, xl = _fp8_split(xT, XS)
        xh, xl = _swz_x(xh), _swz_x(xl)
    else:
        xh = _swz_x(np.ascontiguousarray(xT.astype(bf)))
    in_maps = []
    for i in range(NCORES):
        h0 = HPC * i
        # columns: q_h0 | q_h1 | k_h0 | k_h1
        wqk_i = np.concatenate(
            [W_qkv[(h0 + hh) * D:(h0 + hh + 1) * D, :].T for hh in range(HPC)]
            + [W_qkv[C + (h0 + hh) * D:C + (h0 + hh + 1) * D, :].T
               for hh in range(HPC)],
            axis=1)                                   # [C, HPC*2*D]
        wv_i = W_qkv[2 * C + h0 * D:2 * C + (h0 + HPC) * D, :].T  # [C, HPC*D]
        wp_i = W_proj[:, h0 * D:(h0 + HPC) * D].T     # [HPC*D, C]
        im = {"xh": xh,
              "wp": np.ascontiguousarray(wp_i.astype(bf))}
        if FP8A:
            wqkh, wqkl = _fp8_split(wqk_i, WS)
            wvh, wvl = _fp8_split(wv_i, WS)
            im.update(xl=xl, wqkh=_swz_w(wqkh), wqkl=_swz_w(wqkl),
                      wvh=_swz_w(wvh), wvl=_swz_w(wvl))
        else:
            im.update(wqkh=_swz_w(np.ascontiguousarray(wqk_i.astype(bf))),
                      wvh=_swz_w(np.ascontiguousarray(wv_i.astype(bf))))
        in_maps.append(im)
    return in_maps


def kernel(x, W_qkv, W_proj, b_proj, trace=False):
    global LAST_EXEC_NS
    x = np.ascontiguousarray(np.asarray(x, dtype=np.float32))
    W_qkv = np.asarray(W_qkv, dtype=np.float32)
    W_proj = np.asarray(W_proj, dtype=np.float32)
    b_proj = np.asarray(b_proj, dtype=np.float32)

    in_maps = _make_in_maps(x, W_qkv, W_proj)
    nc = _get_nc()
    res = None
    for attempt in range(3):
        try:
            res = bass_utils.run_bass_kernel_spmd(
                nc, in_maps, core_ids=list(range(NCORES)), trace=trace)
            break
        except Exception:
            # transient "mesh desynced / NRT_EXEC_UNIT_UNRECOVERABLE" errors
            # clear on retry
            if attempt == 2:
                raise
            import time
            time.sleep(5)
    LAST_EXEC_NS = res.exec_time_ns
    acc = res.results[0]["y"].astype(np.float64)
    for i in range(1, NCORES):
        acc += res.results[i]["y"]
    out = (acc + b_proj).astype(np.float32)
    return out.reshape(B, N, C)


def bench(x, W_qkv, W_proj, b_proj, iters=10):
    """Device-resident repeat timing of the NEFF execution.

    Returns (per_iter_ns_blocking, per_iter_ns_pipelined, full output).
    """
    x = np.ascontiguousarray(np.asarray(x, dtype=np.float32))
    in_maps = _make_in_maps(x, np.asarray(W_qkv, dtype=np.float32),
                            np.asarray(W_proj, dtype=np.float32))
    t_block, t_pipe, y_percore = _bench_impl(in_maps, iters=iters)
    acc = y_percore[0].astype(np.float64)
    for i in range(1, NCORES):
        acc += y_percore[i]
    out = (acc + np.asarray(b_proj, dtype=np.float32)).astype(np.float32)
    return t_block, t_pipe, out.reshape(B, N, C)


def _bench_impl(in_maps, iters=10, nc=None):
    import time
    import jax
    from jax.experimental.shard_map import shard_map
    from jax.sharding import Mesh, PartitionSpec, NamedSharding
    from concourse import bass2jax, mybir as mb

    nc = nc or _get_nc()
    bass2jax.install_neuronx_cc_hook()

    partition_name = (nc.partition_id_tensor.name
                      if nc.partition_id_tensor else None)
    in_names, out_names, out_avals, zero_outs = [], [], [], []
    for alloc in nc.m.functions[0].allocations:
        if not isinstance(alloc, mb.MemoryLocationSet):
            continue
        name = alloc.memorylocations[0].name
        if alloc.kind == "ExternalInput":
            if name != partition_name:
                in_names.append(name)
        elif alloc.kind == "ExternalOutput":
            out_names.append(name)
            shape = tuple(alloc.tensor_shape)
            dtype = mb.dt.np(alloc.dtype)
            out_avals.append(jax.core.ShapedArray(shape, dtype))
            zero_outs.append(np.zeros(shape, dtype))
    n_params = len(in_names)
    all_names = in_names + out_names
    if partition_name is not None:
        all_names = all_names + [partition_name]

    def _body(*args):
        operands = list(args)
        if partition_name is not None:
            operands.append(bass2jax.partition_id_tensor())
        outs = bass2jax._bass_exec_p.bind(
            *operands,
            out_avals=tuple(out_avals),
            in_names=tuple(all_names),
            out_names=tuple(out_names),
            lowering_input_output_aliases=(),
            sim_require_finite=True,
            sim_require_nnan=True,
            nc=nc,
        )
        return tuple(outs)

    devices = jax.devices()[:NCORES]
    mesh = Mesh(np.asarray(devices), ("core",))
    spec = PartitionSpec("core")
    sharded = jax.jit(
        shard_map(_body, mesh=mesh,
                  in_specs=(spec,) * (n_params + len(out_names)),
                  out_specs=(spec,) * len(out_names),
                  check_rep=False),
        keep_unused=True)

    shd = NamedSharding(mesh, spec)
    concat_in = [
        np.concatenate([np.asarray(in_maps[c][nm]) for c in range(NCORES)],
                       axis=0) for nm in in_names]
    concat_zero = [np.zeros((NCORES * z.shape[0], *z.shape[1:]), z.dtype)
                   for z in zero_outs]
    dev_in = [jax.device_put(a, shd) for a in concat_in]
    dev_zero = [jax.device_put(a, shd) for a in concat_zero]

    out = sharded(*dev_in, *dev_zero)           # warm-up / compile
    jax.block_until_ready(out)
    if iters == 0:
        return (sharded, dev_in, dev_zero, out_names)

    t_block = []
    for _ in range(iters):
        t0 = time.perf_counter()
        out = sharded(*dev_in, *dev_zero)
        jax.block_until_ready(out)
        t_block.append(time.perf_counter() - t0)

    t0 = time.perf_counter()
    outs = [sharded(*dev_in, *dev_zero) for _ in range(iters)]
    jax.block_until_ready(outs)
    t_pipe = (time.perf_counter() - t0) / iters

    y_global = np.asarray(out[out_names.index("y")])
    return (min(t_block) * 1e9, t_pipe * 1e9,
            y_global.reshape(NCORES, -1, y_global.shape[-1]))


# revision 73
# speedup vs baseline: 1.0056x; 1.0056x over previous
"""Multi-head attention (B=4, N=2048, C=1024, H=16, D=64) on 8 Trainium2 cores.

Sharding: tensor-parallel over heads (2 heads per core); every core reads the
full x (pre-transposed to [C, tokens] bf16 on host) plus its head-slice of the
weights, and writes a full-shape partial projection output.  The 8 partials
are summed on host (the "unshard" of a row-parallel output projection).

v3 layout (372us baseline -> 310us): restructured around the two
near-saturated engines, ACT (exp of 33.5M scores/core) and PE.

  - exp batching: the two heads' score tiles for one key chunk land in one
    2-bank PSUM tile [128, 2, 512] f32, so each ACT instruction processes
    1024 elements (amortizes the fixed SBUF/PSUM access latency); ACT busy
    drops 313us -> ~266us and is the pipeline's pace-setter.
  - AV in the flipped orientation: probs are the *stationary* operand
    (pb[:, hh, 128q-slice]), V' [128k, 65] moves, out = av[128q, 65] f32
    accumulated over the 16 key chunks.  Cost per matmul is the moving free
    size (65) instead of 512, halving AV PE time; the ones-column denominator
    rides along as column 64 (scaled by S2 to match the fp8 product scale).
  - normalization: the denominator is per-query on the PARTITION dim, so a
    [128,1] reciprocal_approx_fast + per-partition tensor_scalar_mul
    replaces the PE broadcast matmul + row shuffle of the baseline.
  - the projection operand attn^T [ch, q] is produced by PE transpose
    (128-cycle instrs) + DVE copies instead of re-orienting AV.
  - phase A runs 3-term split-precision fp8 DoubleRow matmuls
    (x = xh + xl, w = wh + wl, hi*hi + lo*hi + hi*lo at 0.5 PE
    cycles/row), ~25% less PE time than bf16 at slightly BETTER accuracy;
    planes are pre-scaled/pre-swizzled on host so every DMA is one
    contiguous run per partition.
  - emission is software-pipelined per (batch, query-block) slot: slot s
    emits ST/exp of slot s+1 paced 1:1 against everything else, with phase-A
    fill (exp-independent) front-loaded and the exp-dependent
    AV->norm->transpose->projection chain in the back half.  All x/weight
    DMAs ride the SP queue (the scalar queue shares the ACT sequencer and
    would head-block the exp stream); y goes out once per slot on the Pool
    SWDGE queue.

Score/AV/projection matmuls are bf16 (1 PE cycle/row) with fp32 PSUM
accumulation.  Cost-model sim: ~310 us/core (PE ~259us busy, ACT ~267us).
"""

import numpy as np
import ml_dtypes
from contextlib import ExitStack

import concourse.mybir as mybir
import concourse.tile as tile
from concourse import bacc
from concourse import bass_utils
from concourse.masks import make_identity

B, N, C = 4, 2048, 1024
H, D = 16, 64
T = B * N                 # 8192 tokens
NCORES = 8
HPC = H // NCORES         # heads per core = 2
SCALE = D ** -0.5

F32 = mybir.dt.float32
BF16 = mybir.dt.bfloat16

TS = 512                  # phase-A token tile (free dim)
CCN = C // 128            # 8 contraction chunks
KC = N // 128             # 16 key chunks per batch
QB = N // 512             # 4 query blocks per batch
NSLOT = B * QB            # 16 (b, qb) slots


FP8 = mybir.dt.float8e4
FP8A = True               # fp8 split-precision phase A (see phase_A_qk)
XS = 4.0 if FP8A else 1.0  # host-side scale on x planes
WS = 16.0 if FP8A else 1.0  # host-side scale on wqk/wv planes
S2 = XS * WS              # scale carried by qT/kT/v
XDT = FP8 if FP8A else mybir.dt.bfloat16


def _build_graph():
    nc = bacc.Bacc("TRN2", target_bir_lowering=False, debug=False,
                   num_devices=NCORES)
    # x and the qkv weights ship as split-precision fp8 pairs (hi + residual);
    # phase A runs 3-term DoubleRow matmuls (hi*hi + lo*hi + hi*lo) at 2x the
    # bf16 matmul rate with ~bf16 accuracy.  x is scaled by XS and w by WS on
    # host so the residual planes stay in fp8's normal range; the product
    # scale S2 rides on qT/kT/v and is folded into the exp scale and the
    # ones-column (= S2) of V'.
    # all fp8 planes ship pre-swizzled to [partition, ...contiguous] so each
    # DMA is one contiguous run per partition (descriptor overhead killed)
    NTS = T // TS
    xh = nc.dram_tensor("xh", [128, NTS, CCN, TS], XDT, kind="ExternalInput").ap()
    # wqk columns: [q_h0 | q_h1 | k_h0 | k_h1], each D wide
    wqkh = nc.dram_tensor("wqkh", [128, CCN, HPC * 2 * D], XDT, kind="ExternalInput").ap()
    if FP8A:
        xl = nc.dram_tensor("xl", [128, NTS, CCN, TS], FP8, kind="ExternalInput").ap()
        wqkl = nc.dram_tensor("wqkl", [128, CCN, HPC * 2 * D], FP8, kind="ExternalInput").ap()
        wvl = nc.dram_tensor("wvl", [128, CCN, HPC * D], FP8, kind="ExternalInput").ap()
    wvh = nc.dram_tensor("wvh", [128, CCN, HPC * D], XDT, kind="ExternalInput").ap()
    wp = nc.dram_tensor("wp", [HPC * D, C], BF16, kind="ExternalInput").ap()
    y = nc.dram_tensor("y", [T, C], BF16, kind="ExternalOutput").ap()

    with tile.TileContext(nc) as tc, ExitStack() as ctx:
        const = ctx.enter_context(tc.tile_pool(name="const", bufs=1))
        xpool = ctx.enter_context(tc.tile_pool(name="x", bufs=6))
        probs = ctx.enter_context(tc.tile_pool(name="probs", bufs=34))
        attnp = ctx.enter_context(tc.tile_pool(name="attn", bufs=6))
        atTp = ctx.enter_context(tc.tile_pool(name="attnT", bufs=3))
        rpool = ctx.enter_context(tc.tile_pool(name="recip", bufs=4))
        outp = ctx.enter_context(tc.tile_pool(name="out", bufs=2))
        # PSUM: st 2x2 banks + av 2x1 + mm 2x1 = 8 banks
        st_psum = ctx.enter_context(
            tc.tile_pool(name="st", bufs=2, space="PSUM"))
        av_psum = ctx.enter_context(
            tc.tile_pool(name="av", bufs=2, space="PSUM"))
        mm_psum = ctx.enter_context(
            tc.tile_pool(name="mm", bufs=2, space="PSUM"))

        ident = const.tile([128, 128], BF16)
        make_identity(nc, ident[:])
        # burn through the PE p-state ramp while the first DMAs land
        for _ in range(28):
            warm = mm_psum.tile([128, 128], BF16, name="warm", tag="mm")
            nc.tensor.transpose(warm[:], ident[:], ident[:])

        wqkh_sb = const.tile([128, CCN, HPC * 2 * D], XDT)
        wvh_sb = const.tile([128, CCN, HPC * D], XDT)
        if FP8A:
            wqkl_sb = const.tile([128, CCN, HPC * 2 * D], FP8)
            wvl_sb = const.tile([128, CCN, HPC * D], FP8)
        wp_sb = const.tile([128, C], BF16)

        # per-batch phase-A outputs:
        #   qT/kT: [d, tokens] * S2; head hh lives on partitions hh*64..+63
        #   v: per head, 16 key-tiles of [128 tok, 65] * S2 (col 64 = S2)
        qT_b, kT_b, v_b = [], [], []
        for b in range(B):
            qT_b.append(const.tile([128, N], BF16, name=f"qTb{b}", tag=f"qT{b}"))
            kT_b.append(const.tile([128, N], BF16, name=f"kTb{b}", tag=f"kT{b}"))
            v_b.append(const.tile([128, HPC, KC, D + 1], BF16,
                                  name=f"vb{b}", tag=f"v{b}"))
            nc.gpsimd.memset(v_b[b][:, :, :, D:D + 1], S2)

        x_tiles = {}

        def phase_A_dma(b, ts):
            ci = b * 4 + ts
            xt_h = xpool.tile([128, CCN, TS], XDT, name=f"xh{b}_{ts}", tag="xth")
            # all x planes on the SP queue: the scalar queue shares the ACT
            # sequencer and would head-block the exp stream
            nc.sync.dma_start(xt_h[:], xh[:, ci, :, :])
            xt_l = None
            if FP8A:
                xt_l = xpool.tile([128, CCN, TS], FP8,
                                  name=f"xl{b}_{ts}", tag="xtl")
                nc.sync.dma_start(xt_l[:], xl[:, ci, :, :])
            x_tiles[(b, ts)] = (xt_h, xt_l)

        DR = mybir.MatmulPerfMode.DoubleRow

        ps_tiles = {}

        def phase_A_qk(b, ts, qk_i, half=None):
            # dst in {q, k}; emitted as two halves with ONLY an ST between
            # them (STs never allocate from the mm pool, so the open
            # accumulator is safe); half=None runs both
            xt_h, xt_l = x_tiles[(b, ts)]
            dst = (qT_b[b], kT_b[b])[qk_i]
            csl = slice(qk_i * 128, (qk_i + 1) * 128)
            halves = (0, 1) if half is None else (half,)
            if halves[0] == 0:
                ps_tiles[(b, ts, qk_i)] = mm_psum.tile(
                    [128, TS], F32, name=f"ps{b}_{ts}_{qk_i}", tag="mm")
            ps = ps_tiles[(b, ts, qk_i)]
            if FP8A:
                terms = ((xt_h, wqkh_sb), (xt_l, wqkh_sb), (xt_h, wqkl_sb))
                pairs = [(ti, cp) for ti in range(3) for cp in range(4)]
                for hf in halves:
                    for ti, cp in pairs[6 * hf:6 * hf + 6]:
                        xt, wt = terms[ti]
                        nc.tensor.matmul(
                            ps[:],
                            wt[:, 2 * cp:2 * cp + 2, csl],
                            xt[:, 2 * cp:2 * cp + 2, :],
                            start=(ti == 0 and cp == 0),
                            stop=(ti == 2 and cp == 3),
                            perf_mode=DR)
            else:
                for hf in halves:
                    for cc in range(4 * hf, 4 * hf + 4):
                        nc.tensor.matmul(
                            ps[:], wqkh_sb[:, cc, csl], xt_h[:, cc, :],
                            start=(cc == 0), stop=(cc == CCN - 1))
            if halves[-1] == 1:
                ps_tiles.pop((b, ts, qk_i))
                nc.vector.tensor_copy(dst[:, ts * TS:(ts + 1) * TS], ps[:])

        def phase_A_v(b, ts, jh):
            # one of two units per (b, ts): token-subtile pair
            xt_h, xt_l = x_tiles[(b, ts)]
            for j in (2 * jh, 2 * jh + 1):
                jsl = slice(j * 128, (j + 1) * 128)
                vp = mm_psum.tile([128, HPC * D], F32, tag="mm")
                if FP8A:
                    terms = ((xt_h, wvh_sb), (xt_l, wvh_sb), (xt_h, wvl_sb))
                    for ti in range(3):
                        xt, wt = terms[ti]
                        for cp in range(4):
                            nc.tensor.matmul(
                                vp[:],
                                xt[:, 2 * cp:2 * cp + 2, jsl],
                                wt[:, 2 * cp:2 * cp + 2, :],
                                start=(ti == 0 and cp == 0),
                                stop=(ti == 2 and cp == 3),
                                perf_mode=DR)
                else:
                    for cc in range(CCN):
                        nc.tensor.matmul(
                            vp[:], xt_h[:, cc, jsl], wvh_sb[:, cc, :],
                            start=(cc == 0), stop=(cc == CCN - 1))
                for hh in range(HPC):
                    nc.vector.tensor_copy(
                        v_b[b][:, hh, ts * 4 + j, 0:D],
                        vp[:, hh * D:(hh + 1) * D])
            if jh == 1:
                x_tiles.pop((b, ts))

        # ---- phase B building blocks --------------------------------------
        pb_tiles = {}

        def st_exp(s, kc):
            b, qb = divmod(s, QB)
            q0, k0 = qb * 512, kc * 128
            st = st_psum.tile([128, 2, 512], F32, tag="st")
            for hh in range(HPC):
                h0 = hh * 64
                nc.tensor.matmul(
                    st[:, hh, :],
                    kT_b[b][h0:h0 + 64, k0:k0 + 128],
                    qT_b[b][h0:h0 + 64, q0:q0 + 512],
                    start=True, stop=True)
            pb = probs.tile([128, 2, 512], BF16, tag="probs")
            nc.scalar.activation(pb[:], st[:],
                                 mybir.ActivationFunctionType.Exp,
                                 scale=SCALE / (S2 * S2))
            pb_tiles[(s, kc)] = pb

        av_tiles = {}

        def av_group(s, hh, qsub):
            b, qb = divmod(s, QB)
            if qsub == 0:
                av_tiles[(s, hh)] = av_psum.tile([128, 4, D + 1], F32,
                                                 name=f"av{s}_{hh}", tag="av")
            av = av_tiles[(s, hh)]
            for kc in range(KC):
                pb = pb_tiles[(s, kc)]
                nc.tensor.matmul(
                    av[:, qsub, :],
                    pb[:, hh, qsub * 128:(qsub + 1) * 128],
                    v_b[b][:, hh, kc, :],
                    start=(kc == 0), stop=(kc == KC - 1))
            if hh == HPC - 1 and qsub == 3:
                for kc in range(KC):
                    pb_tiles.pop((s, kc))

        def av_kc(s, kc):
            # kc-major variant for the final slot: accumulate all 8 regions
            # per key chunk so AV tracks the exp stream instead of
            # serializing after it
            b, qb = divmod(s, QB)
            if kc == 0:
                for hh in range(HPC):
                    av_tiles[(s, hh)] = av_psum.tile(
                        [128, 4, D + 1], F32, name=f"av{s}_{hh}", tag="av")
            pb = pb_tiles[(s, kc)]
            for hh in range(HPC):
                for qsub in range(4):
                    nc.tensor.matmul(
                        av_tiles[(s, hh)][:, qsub, :],
                        pb[:, hh, qsub * 128:(qsub + 1) * 128],
                        v_b[b][:, hh, kc, :],
                        start=(kc == 0), stop=(kc == KC - 1))
            if kc == KC - 1:
                for k2 in range(KC):
                    pb_tiles.pop((s, k2))

        attnT_tiles = {}
        at_tiles = {}

        def norm_mul(s, qsub):
            # per-query normalization on DVE (denominator = av column 64,
            # carrying the same S2 scale as the numerator)
            if qsub == 0:
                attnT_tiles[s] = atTp.tile([128, 4, 128], BF16,
                                           name=f"aT{s}", tag="attnT")
            rc = rpool.tile([128, 2], F32, name=f"rc{s}_{qsub}", tag="rc")
            at = attnp.tile([128, 128], BF16, tag="attn")
            at_tiles[(s, qsub)] = at
            for hh in range(HPC):
                nc.vector.reciprocal_approx_fast(
                    rc[:, hh:hh + 1], av_tiles[(s, hh)][:, qsub, D:D + 1])
                nc.vector.tensor_scalar_mul(
                    at[:, hh * 64:(hh + 1) * 64],
                    av_tiles[(s, hh)][:, qsub, 0:D],
                    rc[:, hh:hh + 1])

        def norm_T(s, qsub):
            # PE transpose into the projection orientation [ch, q]
            aT = attnT_tiles[s]
            at = at_tiles.pop((s, qsub))
            tp = mm_psum.tile([128, 128], BF16, tag="mm")
            nc.tensor.transpose(tp[:], at[:], ident[:])
            nc.vector.tensor_copy(aT[:, qsub, :], tp[:])

        ot_tiles = {}

        def phase_P(s, qsub):
            b, qb = divmod(s, QB)
            aT = attnT_tiles[s]
            if qsub == 0:
                ot_tiles[s] = outp.tile([128, 4, C], BF16, name=f"ot{s}",
                                        tag="out")
            ot = ot_tiles[s]
            for ob in range(2):
                pp = mm_psum.tile([128, 512], F32, tag="mm")
                nc.tensor.matmul(
                    pp[:],
                    aT[:, qsub, :],
                    wp_sb[:, ob * 512:(ob + 1) * 512],
                    start=True, stop=True)
                nc.vector.tensor_copy(ot[:, qsub, ob * 512:(ob + 1) * 512],
                                      pp[:])
            if qsub == 3:
                # one batched y DMA per slot on the Pool SWDGE queue, keeping
                # the sync HWDGE queue clear for the x streams
                r0 = b * N + qb * 512
                nc.gpsimd.dma_start(
                    y[r0:r0 + 512, :].rearrange("(q p) c -> p q c", p=128),
                    ot[:])
                attnT_tiles.pop(s)
                ot_tiles.pop(s)

        # ---- software-pipelined emission ----------------------------------
        # prologue: batch 0 projections feeding slot 0's ST/exp stream.
        # DMA transfers serialize on one global resource in the cost model,
        # so order them by first use: x chunk 0 and the wqk planes gate the
        # first projection matmul.
        phase_A_dma(0, 0)
        nc.sync.dma_start(wqkh_sb[:], wqkh[:, :, :])
        if FP8A:
            nc.sync.dma_start(wqkl_sb[:], wqkl[:, :, :])
        phase_A_dma(0, 1)
        nc.sync.dma_start(wvh_sb[:], wvh[:, :, :])
        if FP8A:
            nc.sync.dma_start(wvl_sb[:], wvl[:, :, :])
        nc.sync.dma_start(wp_sb[:], wp[:, :])
        # qk-only per ts round (PE ~4.3us vs ACT 4.15us per round); the
        # batch-0 V projections wait until slot 0's fill, where the PE would
        # otherwise idle
        for ts in range(4):
            phase_A_qk(0, ts, 0)
            phase_A_qk(0, ts, 1)
            if ts + 2 < 4:
                phase_A_dma(0, ts + 2)
            st_exp(0, 4 * ts)
            st_exp(0, 4 * ts + 1)
            st_exp(0, 4 * ts + 2)
            st_exp(0, 4 * ts + 3)

        # slot s: ST/exp(s+1) interleaved with the AV->norm->P chain of slot
        # s and phase A of batch b+1.  Emission order IS execution order per
        # engine, so AV groups must follow the A_v writes of their batch and
        # each norm/T must follow both heads' AV groups of its qsub.
        for s in range(NSLOT):
            b, qb = divmod(s, QB)
            nb = b + 1
            sts = ([("st", kc) for kc in range(KC)]
                   if s + 1 < NSLOT else [])
            # per-qsub chain: both heads' AV accumulations, then
            # normalize+transpose, then the output projection; nt/p trail
            # their producers by several units so the PE never stalls on the
            # DVE round-trips (PE executes in emission order)
            chain = [("av", 0, 0), ("av", 1, 0), ("av", 0, 1),
                     ("av", 1, 1), ("nm", s, 0), ("av", 0, 2),
                     ("av", 1, 2), ("nm", s, 1), ("av", 0, 3),
                     ("nt", s, 0), ("av", 1, 3), ("nm", s, 2),
                     ("nt", s, 1), ("p", s, 0), ("nm", s, 3),
                     ("nt", s, 2), ("p", s, 1), ("nt", s, 3),
                     ("p", s, 2), ("p", s, 3)]
            pre = []      # must precede this slot's AV groups
            if s == 0:
                pre += [("av_", 0, ts, jh)
                        for ts in range(4) for jh in range(2)]
            if b >= 1 and qb == 0:
                pre += [("av_", b, 3, 0), ("av_", b, 3, 1)]
            fill = []     # order-flexible PE filler (phase A of next batch)
            if nb < B:
                if qb >= 1:
                    fill += [("av_", nb, qb - 1, 0), ("av_", nb, qb - 1, 1)]
                fill += [("aqk", nb, qb, 0, 0), ("aqk", nb, qb, 0, 1),
                         ("aqk", nb, qb, 1, 0), ("aqk", nb, qb, 1, 1)]
            # Front-load the fill (phase A of the next batch): it does not
            # depend on this slot's exps, so it soaks up the PE idle while
            # ACT drains them; the AV->norm->P chain (which must wait for
            # those exps) runs in the back half, 2 chain units per ST.
            seq = []
            if nb < B:
                seq.append(("dma", nb, qb))
            seq += sts[0:2]
            seq += pre
            si = 2
            for u in fill:
                seq.append(u)
                if si < len(sts):
                    seq.append(sts[si]); si += 1
            ci = 0
            while ci < len(chain) or si < len(sts):
                if ci < len(chain):
                    seq.append(chain[ci]); ci += 1
                if ci < len(chain):
                    seq.append(chain[ci]); ci += 1
                if si < len(sts):
                    seq.append(sts[si]); si += 1

            for u in seq:
                kind = u[0]
                if kind == "st":
                    st_exp(s + 1, u[1])
                elif kind == "av":
                    av_group(s, u[1], u[2])
                elif kind == "avk":
                    av_kc(s, u[1])
                elif kind == "nm":
                    norm_mul(s, u[2])
                elif kind == "nt":
                    norm_T(s, u[2])
                elif kind == "p":
                    phase_P(u[1], u[2])
                elif kind == "dma":
                    phase_A_dma(u[1], u[2])
                elif kind == "aqk":
                    phase_A_qk(u[1], u[2], u[3], u[4])
                elif kind == "av_":
                    phase_A_v(u[1], u[2], u[3])

    nc.compile()
    return nc


_NC = None
LAST_EXEC_NS = None


def _get_nc():
    global _NC
    if _NC is None:
        _NC = _build_graph()
    return _NC


def _fp8_split(a, scale):
    """a*scale ~= hi + lo with both planes fp8e4m3."""
    f8 = ml_dtypes.float8_e4m3
    a = a.astype(np.float32) * scale
    hi = a.astype(f8)
    lo = (a - hi.astype(np.float32)).astype(f8)
    return np.ascontiguousarray(hi), np.ascontiguousarray(lo)


def _swz_x(a):
    # [C, T] -> [128, T//TS, CCN, TS]: per-partition contiguous DMA chunks
    return np.ascontiguousarray(
        a.reshape(CCN, 128, T // TS, TS).transpose(1, 2, 0, 3))


def _swz_w(a):
    # [C, W] -> [128, CCN, W]
    return np.ascontiguousarray(
        a.reshape(CCN, 128, a.shape[1]).transpose(1, 0, 2))


def _make_in_maps(x, W_qkv, W_proj):
    bf = ml_dtypes.bfloat16
    xT = x.reshape(T, C).T
    if FP8A:
        xh, xl = _fp8_split(xT, XS)
        xh, xl = _swz_x(xh), _swz_x(xl)
    else:
        xh = _swz_x(np.ascontiguousarray(xT.astype(bf)))
    in_maps = []
    for i in range(NCORES):
        h0 = HPC * i
        # columns: q_h0 | q_h1 | k_h0 | k_h1
        wqk_i = np.concatenate(
            [W_qkv[(h0 + hh) * D:(h0 + hh + 1) * D, :].T for hh in range(HPC)]
            + [W_qkv[C + (h0 + hh) * D:C + (h0 + hh + 1) * D, :].T
               for hh in range(HPC)],
            axis=1)                                   # [C, HPC*2*D]
        wv_i = W_qkv[2 * C + h0 * D:2 * C + (h0 + HPC) * D, :].T  # [C, HPC*D]
        wp_i = W_proj[:, h0 * D:(h0 + HPC) * D].T     # [HPC*D, C]
        im = {"xh": xh,
              "wp": np.ascontiguousarray(wp_i.astype(bf))}
        if FP8A:
            wqkh, wqkl = _fp8_split(wqk_i, WS)
            wvh, wvl = _fp8_split(wv_i, WS)
            im.update(xl=xl, wqkh=_swz_w(wqkh), wqkl=_swz_w(wqkl),
                      wvh=_swz_w(wvh), wvl=_swz_w(wvl))
        else:
            im.update(wqkh=_swz_w(np.ascontiguousarray(wqk_i.astype(bf))),
                      wvh=_swz_w(np.ascontiguousarray(wv_i.astype(bf))))
        in_maps.append(im)
    return in_maps


def kernel(x, W_qkv, W_proj, b_proj, trace=False):
    global LAST_EXEC_NS
    x = np.ascontiguousarray(np.asarray(x, dtype=np.float32))
    W_qkv = np.asarray(W_qkv, dtype=np.float32)
    W_proj = np.asarray(W_proj, dtype=np.float32)
    b_proj = np.asarray(b_proj, dtype=np.float32)

    in_maps = _make_in_maps(x, W_qkv, W_proj)
    nc = _get_nc()
    res = None
    for attempt in range(3):
        try:
            res = bass_utils.run_bass_kernel_spmd(
                nc, in_maps, core_ids=list(range(NCORES)), trace=trace)
            break
        except Exception:
            # transient "mesh desynced / NRT_EXEC_UNIT_UNRECOVERABLE" errors
            # clear on retry
            if attempt == 2:
                raise
            import time
            time.sleep(5)
    LAST_EXEC_NS = res.exec_time_ns
    acc = res.results[0]["y"].astype(np.float64)
    for i in range(1, NCORES):
        acc += res.results[i]["y"]
    out = (acc + b_proj).astype(np.float32)
    return out.reshape(B, N, C)


def bench(x, W_qkv, W_proj, b_proj, iters=10):
    """Device-resident repeat timing of the NEFF execution.

    Returns (per_iter_ns_blocking, per_iter_ns_pipelined, full output).
    """
    x = np.ascontiguousarray(np.asarray(x, dtype=np.float32))
    in_maps = _make_in_maps(x, np.asarray(W_qkv, dtype=np.float32),
                            np.asarray(W_proj, dtype=np.float32))
    t_block, t_pipe, y_percore = _bench_impl(in_maps, iters=iters)
    acc = y_percore[0].astype(np.float64)
    for i in range(1, NCORES):
        acc += y_percore[i]
    out = (acc + np.asarray(b_proj, dtype=np.float32)).astype(np.float32)
    return t_block, t_pipe, out.reshape(B, N, C)


def _bench_impl(in_maps, iters=10, nc=None):
    import time
    import jax
    from jax.experimental.shard_map import shard_map
    from jax.sharding import Mesh, PartitionSpec, NamedSharding
    from concourse import bass2jax, mybir as mb

    nc = nc or _get_nc()
    bass2jax.install_neuronx_cc_hook()

    partition_name = (nc.partition_id_tensor.name
                      if nc.partition_id_tensor else None)
    in_names, out_names, out_avals, zero_outs = [], [], [], []
    for alloc in nc.m.functions[0].allocations:
        if not isinstance(alloc, mb.MemoryLocationSet):
            continue
        name = alloc.memorylocations[0].name
        if alloc.kind == "ExternalInput":
            if name != partition_name:
                in_names.append(name)
        elif alloc.kind == "ExternalOutput":
            out_names.append(name)
            shape = tuple(alloc.tensor_shape)
            dtype = mb.dt.np(alloc.dtype)
            out_avals.append(jax.core.ShapedArray(shape, dtype))
            zero_outs.append(np.zeros(shape, dtype))
    n_params = len(in_names)
    all_names = in_names + out_names
    if partition_name is not None:
        all_names = all_names + [partition_name]

    def _body(*args):
        operands = list(args)
        if partition_name is not None:
            operands.append(bass2jax.partition_id_tensor())
        outs = bass2jax._bass_exec_p.bind(
            *operands,
            out_avals=tuple(out_avals),
            in_names=tuple(all_names),
            out_names=tuple(out_names),
            lowering_input_output_aliases=(),
            sim_require_finite=True,
            sim_require_nnan=True,
            nc=nc,
        )
        return tuple(outs)

    devices = jax.devices()[:NCORES]
    mesh = Mesh(np.asarray(devices), ("core",))
    spec = PartitionSpec("core")
    sharded = jax.jit(
        shard_map(_body, mesh=mesh,
                  in_specs=(spec,) * (n_params + len(out_names)),
                  out_specs=(spec,) * len(out_names),
                  check_rep=False),
        keep_unused=True)

    shd = NamedSharding(mesh, spec)
    concat_in = [
        np.concatenate([np.asarray(in_maps[c][nm]) for c in range(NCORES)],
                       axis=0) for nm in in_names]
    concat_zero = [np.zeros((NCORES * z.shape[0], *z.shape[1:]), z.dtype)
                   for z in zero_outs]
    dev_in = [jax.device_put(a, shd) for a in concat_in]
    dev_zero = [jax.device_put(a, shd) for a in concat_zero]

    out = sharded(*dev_in, *dev_zero)           # warm-up / compile
    jax.block_until_ready(out)
    if iters == 0:
        return (sharded, dev_in, dev_zero, out_names)

    t_block = []
    for _ in range(iters):
        t0 = time.perf_counter()
        out = sharded(*dev_in, *dev_zero)
        jax.block_until_ready(out)
        t_block.append(time.perf_counter() - t0)

    t0 = time.perf_counter()
    outs = [sharded(*dev_in, *dev_zero) for _ in range(iters)]
    jax.block_until_ready(outs)
    t_pipe = (time.perf_counter() - t0) / iters

    y_global = np.asarray(out[out_names.index("y")])
    return (min(t_block) * 1e9, t_pipe * 1e9,
            y_global.reshape(NCORES, -1, y_global.shape[-1]))
